# revision 12
# baseline (speedup 1.0000x reference)
"""Trainium2 Bass kernel for DeformationTrackerBiFlowModel — G=7 single-matmul.

Reference math (per batch element b, per step t):
    x_t   = [prev_out (2), fin_t (3)]            (5,)
    h_t   = tanh(x_t @ W_rnn + b_rnn)            (12,)   (U_rnn is inert)
    out_t = [cp0 (2), h_t (12)] @ W_out + b_out  (2,)
    prev_out_{t+1} = out_t;  prev_out_0 = cp0

One matmul + one tanh per step per chain. With G=7 trajectories packed
block-diagonally, the contraction stacks h (12G=84 rows) and the input block
(fin 3G=21 + ones 1 + cp0 2G=14 rows): K=120 <= 128. The output packs
pre (84) | pad (12) | out (14): M=110 <= 128, with the out region at psum
base partition 96 (legal engine AP base). Per step:

    psum_t[0:84]   = h_{t-1} @ Wh + [fin_t, 1, cp0] @ (wF part)   -> tanh
    psum_t[96:110] = h_{t-1} @ Wo2                                -> +c -> out_{t-1}

tanh writes h_t directly into the next step's rhs tile (persistent tiles
whose ones/cp0 rows are written once; only fin rows are DMA'd per step).
Step 0 uses w0 (zero h block, direct W1p/b_rnn rows); step T emits only
out_{T-1}.  vs the original: fin is staged in DRAM with each 4-step group
contiguous per partition-row (21 descriptors per prefetch DMA instead of
84), out groups are written to DRAM verbatim from the staging tile (14
descriptors instead of 56; host unscrambles), and the setup DMAs are
chunked across the sync/scalar/gpsimd queues so the ~120 per-partition
weight descriptors stream in parallel.

Batch 65536 over 8 cores; per core G*C*COLS = 7*3*391 = 8211 (8192 + pad 19).
"""

import os
from contextlib import ExitStack

import numpy as np

import concourse.mybir as mybir
import concourse.tile as tile
from concourse import bacc
from concourse.bass_utils import run_bass_kernel_spmd

B, T = 65536, 100
D_CP, D_FIN, HID = 2, 3, 12
NCORES = 8
BC = B // NCORES              # 8192 per core
G = 7                         # trajectories packed per matmul (block-diag)
C = 3                         # independent column chains
COLS = 391                    # batch columns per chain
BP = G * C * COLS             # 8211 padded batch per core
NH = HID * G                  # 84: h rows (rhs) / pre rows (psum)
NFIN = D_FIN * G              # 21 fin rows
NCONST = 1 + D_CP * G         # 15: ones + cp0 rows
KTOT = NH + NFIN + NCONST     # 120
MOUT = D_CP * G               # 14 out rows
MPAD = 96                     # out region starts at psum partition 96
MTOT = MPAD + MOUT            # 110
NFG = (T + 3) // 4 + 1        # 26 fin groups (steps 4g..4g+3, zero padded)

F32 = mybir.dt.float32

_MM_CHOICES = {"bf16": mybir.dt.bfloat16, "f32r": mybir.dt.float32r, "f32": F32}
MM_DTYPE = _MM_CHOICES[os.environ.get("DTB_MM", "bf16")]
MM_NP = mybir.dt.np(MM_DTYPE)

LAST_RESULTS = None  # test.py introspects profiling info from here


def build_program(t_steps=T, g=G, c=C, cols=COLS, mm_dtype=None):
    if mm_dtype is None:
        mm_dtype = MM_DTYPE
    XDT = mm_dtype
    nh, nfin, nconst = HID * g, D_FIN * g, 1 + D_CP * g
    ktot = nh + nfin + nconst
    mout = D_CP * g
    mpad, mtot = MPAD, MPAD + D_CP * g
    nfg = (t_steps + 3) // 4 + 1
    nog = t_steps // 4
    nc = bacc.Bacc(target_bir_lowering=False)

    fin = nc.dram_tensor("fin", [c, nfin, nfg, 4 * cols], XDT, kind="ExternalInput")
    xc = nc.dram_tensor("xc", [c, nconst, 8 * cols], XDT, kind="ExternalInput")
    cb = nc.dram_tensor("cb", [c, mout, cols], F32, kind="ExternalInput")
    w = nc.dram_tensor("w", [ktot, mtot], XDT, kind="ExternalInput")
    w0 = nc.dram_tensor("w0", [ktot, mtot], XDT, kind="ExternalInput")
    out = nc.dram_tensor("out", [nog, c, mout, 4 * cols], XDT, kind="ExternalOutput")

    tanh = mybir.ActivationFunctionType.Tanh

    with tile.TileContext(nc) as tc, ExitStack() as ctx:
        const = ctx.enter_context(tc.tile_pool(name="const", bufs=1))
        xpool = ctx.enter_context(tc.tile_pool(name="xpool", bufs=1))
        opool = ctx.enter_context(tc.tile_pool(name="opool", bufs=3))
        psum = ctx.enter_context(tc.tile_pool(name="psum", bufs=2, space="PSUM"))

        # Weight loads chunked across three queues (120 per-partition
        # descriptors would otherwise stream serially on one ring).
        ws = const.tile([ktot, mtot], XDT, name="ws")
        w0s = const.tile([ktot, mtot], XDT, name="w0s")
        qs = [nc.sync, nc.scalar, nc.scalar]
        for i in range(3):
            r0, r1 = 40 * i, 40 * (i + 1)
            qs[i].dma_start(out=w0s[r0:r1, :], in_=w0[r0:r1, :])
        for i in range(3):
            r0, r1 = 40 * i, 40 * (i + 1)
            qs[(i + 1) % 3].dma_start(out=ws[r0:r1, :], in_=w[r0:r1, :])
        cbs = []
        for ch in range(c):
            cbt = const.tile([mout, cols], F32, tag=f"cb{ch}", name=f"cbs{ch}")
            qs[ch].dma_start(out=cbt, in_=cb[ch])
            cbs.append(cbt)

        # One wide persistent rhs tile per chain with 8 column-blocks
        # (block = t % 8). Rows: [h (ACT) | fin (DMA) | ones+cp0 (once)].
        xtiles = []
        for ch in range(c):
            xt = xpool.tile([ktot, 8 * cols], XDT, tag=f"x{ch}", name=f"x_{ch}")
            nc.vector.memset(xt[0:nh, :], 0)
            qs[(ch + 1) % 3].dma_start(out=xt[nh + nfin :, :], in_=xc[ch])
            for gg in range(2):  # fin groups 0,1 = steps 0..7
                qs[(ch + gg) % 3].dma_start(
                    out=xt[nh : nh + nfin, 4 * gg * cols : (4 * gg + 4) * cols],
                    in_=fin[ch, :, gg, :],
                )
            xtiles.append(xt)

        ostages = [None] * c
        for t in range(t_steps + 1):
            for ch in range(c):
                xt = xtiles[ch]
                blk = t % 8
                p1 = psum.tile([mtot, cols], F32, tag=f"p{ch}", name=f"p_{ch}_{t}")
                nc.tensor.matmul(
                    p1, w0s if t == 0 else ws,
                    xt[:, blk * cols : (blk + 1) * cols], start=True, stop=True,
                )

                if t < t_steps:
                    # h_t goes straight into the next step's rhs block.
                    nb = (t + 1) % 8
                    nc.scalar.activation(
                        xt[0:nh, nb * cols : (nb + 1) * cols], p1[0:nh, :], tanh
                    )
                if t > 0:
                    ob = (t - 1) % 4
                    if ob == 0:
                        ostages[ch] = opool.tile(
                            [mout, 4 * cols], XDT, tag=f"o{ch}", name=f"o_{ch}_{t}"
                        )
                    ost = ostages[ch]
                    nc.vector.tensor_add(
                        ost[:, ob * cols : (ob + 1) * cols], p1[mpad:mtot, :], cbs[ch]
                    )
                    if ob == 3:
                        nc.sync.dma_start(out=out[(t - 4) // 4, ch], in_=ost)
                # Prefetch fin group g = steps 4g..4g+3, ~5 steps ahead.
                if t % 4 == 3:
                    gg = (t + 5) // 4
                    if gg < nfg:
                        b0 = (4 * gg) % 8
                        nc.sync.dma_start(
                            out=xt[nh : nh + nfin, b0 * cols : (b0 + 4) * cols],
                            in_=fin[ch, :, gg, :],
                        )
    nc.compile()
    return nc


def build_packed_weights(W_rnn, W_out, b_rnn, b_out, g=G):
    W_rnn = np.asarray(W_rnn, np.float32)
    W_out = np.asarray(W_out, np.float32)
    b_rnn = np.asarray(b_rnn, np.float32)
    b_out = np.asarray(b_out, np.float32)
    W1p, W1f = W_rnn[:D_CP], W_rnn[D_CP:]
    Wo1, Wo2 = W_out[:D_CP], W_out[D_CP:]
    nh, nfin = HID * g, D_FIN * g
    ktot = nh + nfin + 1 + D_CP * g
    mpad, mtot = MPAD, MPAD + D_CP * g
    ones_row = nh + nfin
    cp0_base = ones_row + 1

    w = np.zeros((ktot, mtot), np.float32)
    w0 = np.zeros((ktot, mtot), np.float32)
    E = Wo1 @ W1p                      # (2, 12) cp0 contribution to next pre
    r = b_rnn + b_out @ W1p            # (12,) ones-row weight (steady state)
    Wh = Wo2 @ W1p                     # (12, 12) h contribution to next pre
    for i in range(g):
        hsl = slice(HID * i, HID * (i + 1))
        osl = slice(mpad + D_CP * i, mpad + D_CP * (i + 1))
        w[hsl, hsl] = Wh
        w[hsl, osl] = Wo2
        w0[hsl, osl] = Wo2             # unused at t=0 (h rows are zero) but harmless
        fsl = slice(nh + D_FIN * i, nh + D_FIN * (i + 1))
        w[fsl, hsl] = W1f
        w0[fsl, hsl] = W1f
        w[ones_row, hsl] = r
        w0[ones_row, hsl] = b_rnn
        csl = slice(cp0_base + D_CP * i, cp0_base + D_CP * (i + 1))
        w[csl, hsl] = E
        w0[csl, hsl] = W1p
    return w, w0


def stage_inputs(cp0, fin, cvec, g=G, c=C, cols=COLS, t_steps=T):
    """Batch-major -> feature-major device layouts (b = ch*(g*cols)+gi*cols+j)."""
    bp = g * c * cols
    bc = cp0.shape[0]
    nfg = (t_steps + 3) // 4 + 1
    F = np.zeros((bp, 4 * nfg, D_FIN), np.float32)
    F[:bc, :t_steps] = fin
    cp0_p = np.zeros((bp, D_CP), np.float32)
    cp0_p[:bc] = cp0
    cv_p = np.zeros((bp, D_CP), np.float32)
    cv_p[:bc] = cvec
    # fin group g covers steps 4g..4g+3, contiguous per partition row.
    fin_d = np.ascontiguousarray(
        F.reshape(c, g, cols, nfg, 4, D_FIN).transpose(0, 1, 5, 3, 4, 2)
    ).reshape(c, D_FIN * g, nfg, 4 * cols)
    xc_d = np.ones((c, 1 + D_CP * g, cols), np.float32)
    xc_d[:, 1:, :] = cp0_p.reshape(c, g, cols, D_CP).transpose(0, 1, 3, 2).reshape(
        c, D_CP * g, cols
    )
    xc_d = np.tile(xc_d, (1, 1, 8))
    cb_d = np.ascontiguousarray(
        cv_p.reshape(c, g, cols, D_CP).transpose(0, 1, 3, 2)
    ).reshape(c, D_CP * g, cols)
    return fin_d, xc_d, cb_d


def unstage_output(out_d, bc, g=G, c=C, cols=COLS, t_steps=T):
    """out_d [T/4, c, 14, 4*cols] (step-within-group along the row) -> batch."""
    bp = g * c * cols
    nog = t_steps // 4
    o = out_d.reshape(nog, c, g, D_CP, 4, cols).transpose(1, 2, 5, 0, 4, 3)
    return np.ascontiguousarray(o).reshape(bp, t_steps, D_CP)[:bc]


def kernel(control_point_input, finger_input, W_rnn, U_rnn, b_rnn, W_out, b_out):
    global LAST_RESULTS
    cp = np.asarray(control_point_input, np.float32)
    fin = np.asarray(finger_input, np.float32)
    W_rnn = np.asarray(W_rnn, np.float32)
    b_rnn = np.asarray(b_rnn, np.float32)
    W_out = np.asarray(W_out, np.float32)
    b_out = np.asarray(b_out, np.float32)

    cp0 = cp[:, 0, :]
    cvec = cp0 @ W_out[:D_CP] + b_out
    w, w0 = build_packed_weights(W_rnn, W_out, b_rnn, b_out)
    w, w0 = (x.astype(MM_NP) for x in (w, w0))

    nc = build_program()
    in_maps = []
    for m in range(NCORES):
        sl = slice(m * BC, (m + 1) * BC)
        fin_d, xc_d, cb_d = stage_inputs(cp0[sl], fin[sl], cvec[sl])
        in_maps.append(
            {"fin": fin_d.astype(MM_NP, copy=False),
             "xc": xc_d.astype(MM_NP, copy=False), "cb": cb_d, "w": w, "w0": w0}
        )

    trace = bool(os.environ.get("DTB_TRACE"))
    res = run_bass_kernel_spmd(
        nc, in_maps, core_ids=list(range(NCORES)), trace=trace
    )
    LAST_RESULTS = res

    outs = [
        unstage_output(np.asarray(res.results[m]["out"], np.float32), BC)
        for m in range(NCORES)
    ]
    return np.concatenate(outs, axis=0)


# revision 13
# speedup vs baseline: 1.1342x; 1.1342x over previous
"""Trainium2 Bass kernel for DeformationTrackerBiFlowModel — G=7 single-matmul.

Reference math (per batch element b, per step t):
    x_t   = [prev_out (2), fin_t (3)]            (5,)
    h_t   = tanh(x_t @ W_rnn + b_rnn)            (12,)   (U_rnn is inert)
    out_t = [cp0 (2), h_t (12)] @ W_out + b_out  (2,)
    prev_out_{t+1} = out_t;  prev_out_0 = cp0

One matmul + one tanh per step per chain. With G=7 trajectories packed
block-diagonally, the contraction stacks h (12G=84 rows) and the input block
(fin 3G=21 + ones 1 + cp0 2G=14 rows): K=120 <= 128. The output packs
pre (84) | pad (12) | out (14): M=110 <= 128, with the out region at psum
base partition 96 (legal engine AP base). Per step:

    psum_t[0:84]   = h_{t-1} @ Wh + [fin_t, 1, cp0] @ (wF part)   -> tanh
    psum_t[96:110] = h_{t-1} @ Wo2                                -> +c -> out_{t-1}

tanh writes h_t directly into the next step's rhs tile (persistent tiles
whose ones/cp0 rows are written once; only fin rows are DMA'd per step).
Step 0 uses w0 (zero h block, direct W1p/b_rnn rows); step T emits only
out_{T-1}.  vs the original: fin is staged in DRAM with each 4-step group
contiguous per partition-row (21 descriptors per prefetch DMA instead of
84), out groups are written to DRAM verbatim from the staging tile (14
descriptors instead of 56; host unscrambles), and the setup DMAs are
chunked across the sync/scalar/gpsimd queues so the ~120 per-partition
weight descriptors stream in parallel.

Batch 65536 over 8 cores; per core G*C*COLS = 7*3*391 = 8211 (8192 + pad 19).
"""

import os
from contextlib import ExitStack

import numpy as np

import concourse.mybir as mybir
import concourse.tile as tile
from concourse import bacc
from concourse.bass_utils import run_bass_kernel_spmd

B, T = 65536, 100
D_CP, D_FIN, HID = 2, 3, 12
NCORES = 8
BC = B // NCORES              # 8192 per core
G = 7                         # trajectories packed per matmul (block-diag)
C = 3                         # independent column chains
COLS = 391                    # batch columns per chain
BP = G * C * COLS             # 8211 padded batch per core
NH = HID * G                  # 84: h rows (rhs) / pre rows (psum)
NFIN = D_FIN * G              # 21 fin rows
NCONST = 1 + D_CP * G         # 15: ones + cp0 rows
KTOT = NH + NFIN + NCONST     # 120
MOUT = D_CP * G               # 14 out rows
MPAD = 96                     # out region starts at psum partition 96
MTOT = MPAD + MOUT            # 110
NFG = (T + 3) // 4 + 1        # 26 fin groups (steps 4g..4g+3, zero padded)

F32 = mybir.dt.float32

_MM_CHOICES = {"bf16": mybir.dt.bfloat16, "f32r": mybir.dt.float32r, "f32": F32}
MM_DTYPE = _MM_CHOICES[os.environ.get("DTB_MM", "bf16")]
MM_NP = mybir.dt.np(MM_DTYPE)

LAST_RESULTS = None  # test.py introspects profiling info from here


def build_program(t_steps=T, g=G, c=C, cols=COLS, mm_dtype=None):
    if mm_dtype is None:
        mm_dtype = MM_DTYPE
    XDT = mm_dtype
    nh, nfin, nconst = HID * g, D_FIN * g, 1 + D_CP * g
    ktot = nh + nfin + nconst
    mout = D_CP * g
    mpad, mtot = MPAD, MPAD + D_CP * g
    nfg = (t_steps + 3) // 4 + 1
    nog = t_steps // 4
    nc = bacc.Bacc(target_bir_lowering=False)

    fin = nc.dram_tensor("fin", [c, nfin, nfg, 4 * cols], XDT, kind="ExternalInput")
    xc = nc.dram_tensor("xc", [c, nconst, 8 * cols], XDT, kind="ExternalInput")
    cb = nc.dram_tensor("cb", [c, mout, cols], F32, kind="ExternalInput")
    w = nc.dram_tensor("w", [ktot, mtot], XDT, kind="ExternalInput")
    w0 = nc.dram_tensor("w0", [ktot, mtot], XDT, kind="ExternalInput")
    out = nc.dram_tensor("out", [nog, c, mout, 4 * cols], XDT, kind="ExternalOutput")

    tanh = mybir.ActivationFunctionType.Tanh

    with tile.TileContext(nc) as tc, ExitStack() as ctx:
        const = ctx.enter_context(tc.tile_pool(name="const", bufs=1))
        xpool = ctx.enter_context(tc.tile_pool(name="xpool", bufs=1))
        opool = ctx.enter_context(tc.tile_pool(name="opool", bufs=3))
        psum = ctx.enter_context(tc.tile_pool(name="psum", bufs=2, space="PSUM"))

        # Weight loads chunked across three queues (120 per-partition
        # descriptors would otherwise stream serially on one ring).
        ws = const.tile([ktot, mtot], XDT, name="ws")
        w0s = const.tile([ktot, mtot], XDT, name="w0s")
        qs = [nc.sync, nc.scalar, nc.gpsimd]
        for i in range(3):
            r0, r1 = 40 * i, 40 * (i + 1)
            qs[i].dma_start(out=w0s[r0:r1, :], in_=w0[r0:r1, :])
        for i in range(3):
            r0, r1 = 40 * i, 40 * (i + 1)
            qs[(i + 1) % 3].dma_start(out=ws[r0:r1, :], in_=w[r0:r1, :])
        cbs = []
        for ch in range(c):
            cbt = const.tile([mout, cols], F32, tag=f"cb{ch}", name=f"cbs{ch}")
            qs[ch].dma_start(out=cbt, in_=cb[ch])
            cbs.append(cbt)

        # One wide persistent rhs tile per chain with 8 column-blocks
        # (block = t % 8). Rows: [h (ACT) | fin (DMA) | ones+cp0 (once)].
        xtiles = []
        for ch in range(c):
            xt = xpool.tile([ktot, 8 * cols], XDT, tag=f"x{ch}", name=f"x_{ch}")
            nc.vector.memset(xt[0:nh, :], 0)
            qs[(ch + 1) % 3].dma_start(out=xt[nh + nfin :, :], in_=xc[ch])
            for gg in range(2):  # fin groups 0,1 = steps 0..7
                qs[(ch + gg) % 3].dma_start(
                    out=xt[nh : nh + nfin, 4 * gg * cols : (4 * gg + 4) * cols],
                    in_=fin[ch, :, gg, :],
                )
            xtiles.append(xt)

        ostages = [None] * c
        for t in range(t_steps + 1):
            for ch in range(c):
                xt = xtiles[ch]
                blk = t % 8
                p1 = psum.tile([mtot, cols], F32, tag=f"p{ch}", name=f"p_{ch}_{t}")
                nc.tensor.matmul(
                    p1, w0s if t == 0 else ws,
                    xt[:, blk * cols : (blk + 1) * cols], start=True, stop=True,
                )

                if t < t_steps:
                    # h_t goes straight into the next step's rhs block.
                    nb = (t + 1) % 8
                    nc.scalar.activation(
                        xt[0:nh, nb * cols : (nb + 1) * cols], p1[0:nh, :], tanh
                    )
                if t > 0:
                    ob = (t - 1) % 4
                    if ob == 0:
                        ostages[ch] = opool.tile(
                            [mout, 4 * cols], XDT, tag=f"o{ch}", name=f"o_{ch}_{t}"
                        )
                    ost = ostages[ch]
                    nc.vector.tensor_add(
                        ost[:, ob * cols : (ob + 1) * cols], p1[mpad:mtot, :], cbs[ch]
                    )
                    if ob == 3:
                        nc.gpsimd.dma_start(out=out[(t - 4) // 4, ch], in_=ost)
                # Prefetch fin group g = steps 4g..4g+3, ~5 steps ahead.
                if t % 4 == 3:
                    gg = (t + 5) // 4
                    if gg < nfg:
                        b0 = (4 * gg) % 8
                        nc.sync.dma_start(
                            out=xt[nh : nh + nfin, b0 * cols : (b0 + 4) * cols],
                            in_=fin[ch, :, gg, :],
                        )
    nc.compile()
    return nc


def build_packed_weights(W_rnn, W_out, b_rnn, b_out, g=G):
    W_rnn = np.asarray(W_rnn, np.float32)
    W_out = np.asarray(W_out, np.float32)
    b_rnn = np.asarray(b_rnn, np.float32)
    b_out = np.asarray(b_out, np.float32)
    W1p, W1f = W_rnn[:D_CP], W_rnn[D_CP:]
    Wo1, Wo2 = W_out[:D_CP], W_out[D_CP:]
    nh, nfin = HID * g, D_FIN * g
    ktot = nh + nfin + 1 + D_CP * g
    mpad, mtot = MPAD, MPAD + D_CP * g
    ones_row = nh + nfin
    cp0_base = ones_row + 1

    w = np.zeros((ktot, mtot), np.float32)
    w0 = np.zeros((ktot, mtot), np.float32)
    E = Wo1 @ W1p                      # (2, 12) cp0 contribution to next pre
    r = b_rnn + b_out @ W1p            # (12,) ones-row weight (steady state)
    Wh = Wo2 @ W1p                     # (12, 12) h contribution to next pre
    for i in range(g):
        hsl = slice(HID * i, HID * (i + 1))
        osl = slice(mpad + D_CP * i, mpad + D_CP * (i + 1))
        w[hsl, hsl] = Wh
        w[hsl, osl] = Wo2
        w0[hsl, osl] = Wo2             # unused at t=0 (h rows are zero) but harmless
        fsl = slice(nh + D_FIN * i, nh + D_FIN * (i + 1))
        w[fsl, hsl] = W1f
        w0[fsl, hsl] = W1f
        w[ones_row, hsl] = r
        w0[ones_row, hsl] = b_rnn
        csl = slice(cp0_base + D_CP * i, cp0_base + D_CP * (i + 1))
        w[csl, hsl] = E
        w0[csl, hsl] = W1p
    return w, w0


def stage_inputs(cp0, fin, cvec, g=G, c=C, cols=COLS, t_steps=T):
    """Batch-major -> feature-major device layouts (b = ch*(g*cols)+gi*cols+j)."""
    bp = g * c * cols
    bc = cp0.shape[0]
    nfg = (t_steps + 3) // 4 + 1
    F = np.zeros((bp, 4 * nfg, D_FIN), np.float32)
    F[:bc, :t_steps] = fin
    cp0_p = np.zeros((bp, D_CP), np.float32)
    cp0_p[:bc] = cp0
    cv_p = np.zeros((bp, D_CP), np.float32)
    cv_p[:bc] = cvec
    # fin group g covers steps 4g..4g+3, contiguous per partition row.
    fin_d = np.ascontiguousarray(
        F.reshape(c, g, cols, nfg, 4, D_FIN).transpose(0, 1, 5, 3, 4, 2)
    ).reshape(c, D_FIN * g, nfg, 4 * cols)
    xc_d = np.ones((c, 1 + D_CP * g, cols), np.float32)
    xc_d[:, 1:, :] = cp0_p.reshape(c, g, cols, D_CP).transpose(0, 1, 3, 2).reshape(
        c, D_CP * g, cols
    )
    xc_d = np.tile(xc_d, (1, 1, 8))
    cb_d = np.ascontiguousarray(
        cv_p.reshape(c, g, cols, D_CP).transpose(0, 1, 3, 2)
    ).reshape(c, D_CP * g, cols)
    return fin_d, xc_d, cb_d


def unstage_output(out_d, bc, g=G, c=C, cols=COLS, t_steps=T):
    """out_d [T/4, c, 14, 4*cols] (step-within-group along the row) -> batch."""
    bp = g * c * cols
    nog = t_steps // 4
    o = out_d.reshape(nog, c, g, D_CP, 4, cols).transpose(1, 2, 5, 0, 4, 3)
    return np.ascontiguousarray(o).reshape(bp, t_steps, D_CP)[:bc]


def kernel(control_point_input, finger_input, W_rnn, U_rnn, b_rnn, W_out, b_out):
    global LAST_RESULTS
    cp = np.asarray(control_point_input, np.float32)
    fin = np.asarray(finger_input, np.float32)
    W_rnn = np.asarray(W_rnn, np.float32)
    b_rnn = np.asarray(b_rnn, np.float32)
    W_out = np.asarray(W_out, np.float32)
    b_out = np.asarray(b_out, np.float32)

    cp0 = cp[:, 0, :]
    cvec = cp0 @ W_out[:D_CP] + b_out
    w, w0 = build_packed_weights(W_rnn, W_out, b_rnn, b_out)
    w, w0 = (x.astype(MM_NP) for x in (w, w0))

    nc = build_program()
    in_maps = []
    for m in range(NCORES):
        sl = slice(m * BC, (m + 1) * BC)
        fin_d, xc_d, cb_d = stage_inputs(cp0[sl], fin[sl], cvec[sl])
        in_maps.append(
            {"fin": fin_d.astype(MM_NP, copy=False),
             "xc": xc_d.astype(MM_NP, copy=False), "cb": cb_d, "w": w, "w0": w0}
        )

    trace = bool(os.environ.get("DTB_TRACE"))
    res = run_bass_kernel_spmd(
        nc, in_maps, core_ids=list(range(NCORES)), trace=trace
    )
    LAST_RESULTS = res

    outs = [
        unstage_output(np.asarray(res.results[m]["out"], np.float32), BC)
        for m in range(NCORES)
    ]
    return np.concatenate(outs, axis=0)


# revision 14
# speedup vs baseline: 1.1503x; 1.0142x over previous
"""Trainium2 Bass kernel for DeformationTrackerBiFlowModel — G=7 single-matmul.

Reference math (per batch element b, per step t):
    x_t   = [prev_out (2), fin_t (3)]            (5,)
    h_t   = tanh(x_t @ W_rnn + b_rnn)            (12,)   (U_rnn is inert)
    out_t = [cp0 (2), h_t (12)] @ W_out + b_out  (2,)
    prev_out_{t+1} = out_t;  prev_out_0 = cp0

One matmul + one tanh per step per chain. With G=7 trajectories packed
block-diagonally, the contraction stacks h (12G=84 rows) and the input block
(fin 3G=21 + ones 1 + cp0 2G=14 rows): K=120 <= 128. The output packs
pre (84) | pad (12) | out (14): M=110 <= 128, with the out region at psum
base partition 96 (legal engine AP base). Per step:

    psum_t[0:84]   = h_{t-1} @ Wh + [fin_t, 1, cp0] @ (wF part)   -> tanh
    psum_t[96:110] = h_{t-1} @ Wo2                                -> +c -> out_{t-1}

tanh writes h_t directly into the next step's rhs tile (persistent tiles
whose ones/cp0 rows are written once; only fin rows are DMA'd per step).
Step 0 uses w0 (zero h block, direct W1p/b_rnn rows); step T emits only
out_{T-1}.  vs the original: fin is staged in DRAM with each 4-step group
contiguous per partition-row (21 descriptors per prefetch DMA instead of
84), out groups are written to DRAM verbatim from the staging tile (14
descriptors instead of 56; host unscrambles), and the setup DMAs are
chunked across the sync/scalar/gpsimd queues so the ~120 per-partition
weight descriptors stream in parallel.

Batch 65536 over 8 cores; per core G*C*COLS = 7*3*391 = 8211 (8192 + pad 19).
"""

import os
from contextlib import ExitStack

import numpy as np

import concourse.mybir as mybir
import concourse.tile as tile
from concourse import bacc
from concourse.bass_utils import run_bass_kernel_spmd

B, T = 65536, 100
D_CP, D_FIN, HID = 2, 3, 12
NCORES = 8
BC = B // NCORES              # 8192 per core
G = 7                         # trajectories packed per matmul (block-diag)
C = 3                         # independent column chains
COLS = 391                    # batch columns per chain
BP = G * C * COLS             # 8211 padded batch per core
NH = HID * G                  # 84: h rows (rhs) / pre rows (psum)
NFIN = D_FIN * G              # 21 fin rows
NCONST = 1 + D_CP * G         # 15: ones + cp0 rows
KTOT = NH + NFIN + NCONST     # 120
MOUT = D_CP * G               # 14 out rows
MPAD = 96                     # out region starts at psum partition 96
MTOT = MPAD + MOUT            # 110
NFG = (T + 3) // 4 + 1        # 26 fin groups (steps 4g..4g+3, zero padded)

F32 = mybir.dt.float32

_MM_CHOICES = {"bf16": mybir.dt.bfloat16, "f32r": mybir.dt.float32r, "f32": F32}
MM_DTYPE = _MM_CHOICES[os.environ.get("DTB_MM", "bf16")]
MM_NP = mybir.dt.np(MM_DTYPE)

LAST_RESULTS = None  # test.py introspects profiling info from here


def build_program(t_steps=T, g=G, c=C, cols=COLS, mm_dtype=None):
    if mm_dtype is None:
        mm_dtype = MM_DTYPE
    XDT = mm_dtype
    nh, nfin, nconst = HID * g, D_FIN * g, 1 + D_CP * g
    ktot = nh + nfin + nconst
    mout = D_CP * g
    mpad, mtot = MPAD, MPAD + D_CP * g
    nfg = (t_steps + 3) // 4 + 1
    nog = t_steps // 4
    nc = bacc.Bacc(target_bir_lowering=False)

    fin = nc.dram_tensor("fin", [c, nfin, nfg, 4 * cols], XDT, kind="ExternalInput")
    xc = nc.dram_tensor("xc", [c, nconst, 8 * cols], XDT, kind="ExternalInput")
    cb = nc.dram_tensor("cb", [c, mout, cols], F32, kind="ExternalInput")
    w = nc.dram_tensor("w", [ktot, mtot], XDT, kind="ExternalInput")
    w0 = nc.dram_tensor("w0", [ktot, mtot], XDT, kind="ExternalInput")
    out = nc.dram_tensor("out", [nog, c, mout, 4 * cols], XDT, kind="ExternalOutput")

    tanh = mybir.ActivationFunctionType.Tanh

    with tile.TileContext(nc) as tc, ExitStack() as ctx:
        const = ctx.enter_context(tc.tile_pool(name="const", bufs=1))
        xpool = ctx.enter_context(tc.tile_pool(name="xpool", bufs=1))
        opool = ctx.enter_context(tc.tile_pool(name="opool", bufs=3))
        psum = ctx.enter_context(tc.tile_pool(name="psum", bufs=2, space="PSUM"))

        # Step-0-critical loads first (w0 chunks, block-0 of the rhs tiles),
        # chunked across three queues; bulk loads (ws, cb, blocks 1-7)
        # follow.  Only block 0 needs the h-row memset: every other block's
        # h region is ACT-written before any matmul reads it.
        ws = const.tile([ktot, mtot], XDT, name="ws")
        w0s = const.tile([ktot, mtot], XDT, name="w0s")
        qs = [nc.sync, nc.scalar, nc.gpsimd]
        for i in range(3):
            r0, r1 = 40 * i, 40 * (i + 1)
            qs[i].dma_start(out=w0s[r0:r1, :], in_=w0[r0:r1, :])
        xtiles = []
        for ch in range(c):
            xt = xpool.tile([ktot, 8 * cols], XDT, tag=f"x{ch}", name=f"x_{ch}")
            nc.vector.memset(xt[0:nh, 0:cols], 0)
            qs[ch].dma_start(
                out=xt[nh + nfin :, 0:cols], in_=xc[ch][:, 0:cols]
            )
            qs[(ch + 1) % 3].dma_start(
                out=xt[nh : nh + nfin, 0:cols], in_=fin[ch, :, 0, 0:cols]
            )
            xtiles.append(xt)
        for i in range(3):
            r0, r1 = 40 * i, 40 * (i + 1)
            qs[(i + 1) % 3].dma_start(out=ws[r0:r1, :], in_=w[r0:r1, :])
        cbs = []
        for ch in range(c):
            cbt = const.tile([mout, cols], F32, tag=f"cb{ch}", name=f"cbs{ch}")
            qs[ch].dma_start(out=cbt, in_=cb[ch])
            cbs.append(cbt)
        for ch in range(c):
            xt = xtiles[ch]
            qs[(ch + 2) % 3].dma_start(
                out=xt[nh + nfin :, cols:], in_=xc[ch][:, cols:]
            )
            qs[ch].dma_start(
                out=xt[nh : nh + nfin, cols : 4 * cols],
                in_=fin[ch, :, 0, cols:],
            )
            qs[(ch + 1) % 3].dma_start(
                out=xt[nh : nh + nfin, 4 * cols : 8 * cols], in_=fin[ch, :, 1, :]
            )

        ostages = [None] * c
        for t in range(t_steps + 1):
            for ch in range(c):
                xt = xtiles[ch]
                blk = t % 8
                p1 = psum.tile([mtot, cols], F32, tag=f"p{ch}", name=f"p_{ch}_{t}")
                nc.tensor.matmul(
                    p1, w0s if t == 0 else ws,
                    xt[:, blk * cols : (blk + 1) * cols], start=True, stop=True,
                )

                if t < t_steps:
                    # h_t goes straight into the next step's rhs block.
                    nb = (t + 1) % 8
                    nc.scalar.activation(
                        xt[0:nh, nb * cols : (nb + 1) * cols], p1[0:nh, :], tanh
                    )
                if t > 0:
                    ob = (t - 1) % 4
                    if ob == 0:
                        ostages[ch] = opool.tile(
                            [mout, 4 * cols], XDT, tag=f"o{ch}", name=f"o_{ch}_{t}"
                        )
                    ost = ostages[ch]
                    nc.vector.tensor_add(
                        ost[:, ob * cols : (ob + 1) * cols], p1[mpad:mtot, :], cbs[ch]
                    )
                    if ob == 3:
                        nc.gpsimd.dma_start(out=out[(t - 4) // 4, ch], in_=ost)
                # Prefetch fin group g = steps 4g..4g+3, ~5 steps ahead.
                if t % 4 == 3:
                    gg = (t + 5) // 4
                    if gg < nfg:
                        b0 = (4 * gg) % 8
                        nc.sync.dma_start(
                            out=xt[nh : nh + nfin, b0 * cols : (b0 + 4) * cols],
                            in_=fin[ch, :, gg, :],
                        )
    nc.compile()
    return nc


def build_packed_weights(W_rnn, W_out, b_rnn, b_out, g=G):
    W_rnn = np.asarray(W_rnn, np.float32)
    W_out = np.asarray(W_out, np.float32)
    b_rnn = np.asarray(b_rnn, np.float32)
    b_out = np.asarray(b_out, np.float32)
    W1p, W1f = W_rnn[:D_CP], W_rnn[D_CP:]
    Wo1, Wo2 = W_out[:D_CP], W_out[D_CP:]
    nh, nfin = HID * g, D_FIN * g
    ktot = nh + nfin + 1 + D_CP * g
    mpad, mtot = MPAD, MPAD + D_CP * g
    ones_row = nh + nfin
    cp0_base = ones_row + 1

    w = np.zeros((ktot, mtot), np.float32)
    w0 = np.zeros((ktot, mtot), np.float32)
    E = Wo1 @ W1p                      # (2, 12) cp0 contribution to next pre
    r = b_rnn + b_out @ W1p            # (12,) ones-row weight (steady state)
    Wh = Wo2 @ W1p                     # (12, 12) h contribution to next pre
    for i in range(g):
        hsl = slice(HID * i, HID * (i + 1))
        osl = slice(mpad + D_CP * i, mpad + D_CP * (i + 1))
        w[hsl, hsl] = Wh
        w[hsl, osl] = Wo2
        w0[hsl, osl] = Wo2             # unused at t=0 (h rows are zero) but harmless
        fsl = slice(nh + D_FIN * i, nh + D_FIN * (i + 1))
        w[fsl, hsl] = W1f
        w0[fsl, hsl] = W1f
        w[ones_row, hsl] = r
        w0[ones_row, hsl] = b_rnn
        csl = slice(cp0_base + D_CP * i, cp0_base + D_CP * (i + 1))
        w[csl, hsl] = E
        w0[csl, hsl] = W1p
    return w, w0


def stage_inputs(cp0, fin, cvec, g=G, c=C, cols=COLS, t_steps=T):
    """Batch-major -> feature-major device layouts (b = ch*(g*cols)+gi*cols+j)."""
    bp = g * c * cols
    bc = cp0.shape[0]
    nfg = (t_steps + 3) // 4 + 1
    F = np.zeros((bp, 4 * nfg, D_FIN), np.float32)
    F[:bc, :t_steps] = fin
    cp0_p = np.zeros((bp, D_CP), np.float32)
    cp0_p[:bc] = cp0
    cv_p = np.zeros((bp, D_CP), np.float32)
    cv_p[:bc] = cvec
    # fin group g covers steps 4g..4g+3, contiguous per partition row.
    fin_d = np.ascontiguousarray(
        F.reshape(c, g, cols, nfg, 4, D_FIN).transpose(0, 1, 5, 3, 4, 2)
    ).reshape(c, D_FIN * g, nfg, 4 * cols)
    xc_d = np.ones((c, 1 + D_CP * g, cols), np.float32)
    xc_d[:, 1:, :] = cp0_p.reshape(c, g, cols, D_CP).transpose(0, 1, 3, 2).reshape(
        c, D_CP * g, cols
    )
    xc_d = np.tile(xc_d, (1, 1, 8))
    cb_d = np.ascontiguousarray(
        cv_p.reshape(c, g, cols, D_CP).transpose(0, 1, 3, 2)
    ).reshape(c, D_CP * g, cols)
    return fin_d, xc_d, cb_d


def unstage_output(out_d, bc, g=G, c=C, cols=COLS, t_steps=T):
    """out_d [T/4, c, 14, 4*cols] (step-within-group along the row) -> batch."""
    bp = g * c * cols
    nog = t_steps // 4
    o = out_d.reshape(nog, c, g, D_CP, 4, cols).transpose(1, 2, 5, 0, 4, 3)
    return np.ascontiguousarray(o).reshape(bp, t_steps, D_CP)[:bc]


def kernel(control_point_input, finger_input, W_rnn, U_rnn, b_rnn, W_out, b_out):
    global LAST_RESULTS
    cp = np.asarray(control_point_input, np.float32)
    fin = np.asarray(finger_input, np.float32)
    W_rnn = np.asarray(W_rnn, np.float32)
    b_rnn = np.asarray(b_rnn, np.float32)
    W_out = np.asarray(W_out, np.float32)
    b_out = np.asarray(b_out, np.float32)

    cp0 = cp[:, 0, :]
    cvec = cp0 @ W_out[:D_CP] + b_out
    w, w0 = build_packed_weights(W_rnn, W_out, b_rnn, b_out)
    w, w0 = (x.astype(MM_NP) for x in (w, w0))

    nc = build_program()
    in_maps = []
    for m in range(NCORES):
        sl = slice(m * BC, (m + 1) * BC)
        fin_d, xc_d, cb_d = stage_inputs(cp0[sl], fin[sl], cvec[sl])
        in_maps.append(
            {"fin": fin_d.astype(MM_NP, copy=False),
             "xc": xc_d.astype(MM_NP, copy=False), "cb": cb_d, "w": w, "w0": w0}
        )

    trace = bool(os.environ.get("DTB_TRACE"))
    res = run_bass_kernel_spmd(
        nc, in_maps, core_ids=list(range(NCORES)), trace=trace
    )
    LAST_RESULTS = res

    outs = [
        unstage_output(np.asarray(res.results[m]["out"], np.float32), BC)
        for m in range(NCORES)
    ]
    return np.concatenate(outs, axis=0)


# revision 15
# speedup vs baseline: 1.1543x; 1.0035x over previous
"""Trainium2 Bass kernel for DeformationTrackerBiFlowModel — G=7 single-matmul.

Reference math (per batch element b, per step t):
    x_t   = [prev_out (2), fin_t (3)]            (5,)
    h_t   = tanh(x_t @ W_rnn + b_rnn)            (12,)   (U_rnn is inert)
    out_t = [cp0 (2), h_t (12)] @ W_out + b_out  (2,)
    prev_out_{t+1} = out_t;  prev_out_0 = cp0

One matmul + one tanh per step per chain. With G=7 trajectories packed
block-diagonally, the contraction stacks h (12G=84 rows) and the input block
(fin 3G=21 + ones 1 + cp0 2G=14 rows): K=120 <= 128. The output packs
pre (84) | pad (12) | out (14): M=110 <= 128, with the out region at psum
base partition 96 (legal engine AP base). Per step:

    psum_t[0:84]   = h_{t-1} @ Wh + [fin_t, 1, cp0] @ (wF part)   -> tanh
    psum_t[96:110] = h_{t-1} @ Wo2                                -> +c -> out_{t-1}

tanh writes h_t directly into the next step's rhs tile (persistent tiles
whose ones/cp0 rows are written once; only fin rows are DMA'd per step).
Step 0 uses w0 (zero h block, direct W1p/b_rnn rows); step T emits only
out_{T-1}.  vs the original: fin is staged in DRAM with each 4-step group
contiguous per partition-row (21 descriptors per prefetch DMA instead of
84), out groups are written to DRAM verbatim from the staging tile (14
descriptors instead of 56; host unscrambles), and the setup DMAs are
chunked across the sync/scalar/gpsimd queues so the ~120 per-partition
weight descriptors stream in parallel.

Batch 65536 over 8 cores; per core G*C*COLS = 7*3*391 = 8211 (8192 + pad 19).
"""

import os
from contextlib import ExitStack

import numpy as np

import concourse.mybir as mybir
import concourse.tile as tile
from concourse import bacc
from concourse.bass_utils import run_bass_kernel_spmd

B, T = 65536, 100
D_CP, D_FIN, HID = 2, 3, 12
NCORES = 8
BC = B // NCORES              # 8192 per core
G = 7                         # trajectories packed per matmul (block-diag)
C = 3                         # independent column chains
COLS = 391                    # batch columns per chain
BP = G * C * COLS             # 8211 padded batch per core
NH = HID * G                  # 84: h rows (rhs) / pre rows (psum)
NFIN = D_FIN * G              # 21 fin rows
NCONST = 1 + D_CP * G         # 15: ones + cp0 rows
KTOT = NH + NFIN + NCONST     # 120
MOUT = D_CP * G               # 14 out rows
MPAD = 96                     # out region starts at psum partition 96
MTOT = MPAD + MOUT            # 110
NFG = (T + 3) // 4 + 1        # 26 fin groups (steps 4g..4g+3, zero padded)

F32 = mybir.dt.float32

_MM_CHOICES = {"bf16": mybir.dt.bfloat16, "f32r": mybir.dt.float32r, "f32": F32}
MM_DTYPE = _MM_CHOICES[os.environ.get("DTB_MM", "bf16")]
MM_NP = mybir.dt.np(MM_DTYPE)

LAST_RESULTS = None  # test.py introspects profiling info from here


def build_program(t_steps=T, g=G, c=C, cols=COLS, mm_dtype=None):
    if mm_dtype is None:
        mm_dtype = MM_DTYPE
    XDT = mm_dtype
    nh, nfin, nconst = HID * g, D_FIN * g, 1 + D_CP * g
    ktot = nh + nfin + nconst
    mout = D_CP * g
    mpad, mtot = MPAD, MPAD + D_CP * g
    nfg = (t_steps + 3) // 4 + 1
    nog = t_steps // 4
    nc = bacc.Bacc(target_bir_lowering=False)

    fin = nc.dram_tensor("fin", [c, nfin, nfg, 4 * cols], XDT, kind="ExternalInput")
    xc = nc.dram_tensor("xc", [c, nconst, 8 * cols], XDT, kind="ExternalInput")
    cb = nc.dram_tensor("cb", [c, mout, cols], F32, kind="ExternalInput")
    w = nc.dram_tensor("w", [ktot, mtot], XDT, kind="ExternalInput")
    w0 = nc.dram_tensor("w0", [ktot, mtot], XDT, kind="ExternalInput")
    out = nc.dram_tensor("out", [nog, c, mout, 4 * cols], XDT, kind="ExternalOutput")

    tanh = mybir.ActivationFunctionType.Tanh

    with tile.TileContext(nc) as tc, ExitStack() as ctx:
        const = ctx.enter_context(tc.tile_pool(name="const", bufs=1))
        xpool = ctx.enter_context(tc.tile_pool(name="xpool", bufs=1))
        opool = ctx.enter_context(tc.tile_pool(name="opool", bufs=3))
        psum = ctx.enter_context(tc.tile_pool(name="psum", bufs=2, space="PSUM"))

        # Step-0-critical loads first (w0 chunks, block-0 of the rhs tiles),
        # chunked across three queues; bulk loads (ws, cb, blocks 1-7)
        # follow.  Only block 0 needs the h-row memset: every other block's
        # h region is ACT-written before any matmul reads it.
        ws = const.tile([ktot, mtot], XDT, name="ws")
        w0s = const.tile([ktot, mtot], XDT, name="w0s")
        qs = [nc.sync, nc.scalar, nc.gpsimd]
        for i in range(3):
            r0, r1 = 40 * i, 40 * (i + 1)
            qs[i].dma_start(out=w0s[r0:r1, :], in_=w0[r0:r1, :])
        xtiles = []
        for ch in range(c):
            xt = xpool.tile([ktot, 8 * cols], XDT, tag=f"x{ch}", name=f"x_{ch}")
            nc.vector.memset(xt[0:nh, 0:cols], 0)
            qs[ch].dma_start(
                out=xt[nh + nfin :, 0:cols], in_=xc[ch][:, 0:cols]
            )
            qs[(ch + 1) % 3].dma_start(
                out=xt[nh : nh + nfin, 0:cols], in_=fin[ch, :, 0, 0:cols]
            )
            xtiles.append(xt)
        for i in range(3):
            r0, r1 = 40 * i, 40 * (i + 1)
            qs[(i + 1) % 3].dma_start(out=ws[r0:r1, :], in_=w[r0:r1, :])
        # blocks 1-3 (needed from t=1) before the bulk of blocks 4-7
        for ch in range(c):
            xt = xtiles[ch]
            qs[(ch + 2) % 3].dma_start(
                out=xt[nh + nfin :, cols : 4 * cols],
                in_=xc[ch][:, cols : 4 * cols],
            )
            qs[ch].dma_start(
                out=xt[nh : nh + nfin, cols : 4 * cols],
                in_=fin[ch, :, 0, cols:],
            )
        cbs = []
        for ch in range(c):
            cbt = const.tile([mout, cols], F32, tag=f"cb{ch}", name=f"cbs{ch}")
            qs[ch].dma_start(out=cbt, in_=cb[ch])
            cbs.append(cbt)
        for ch in range(c):
            xt = xtiles[ch]
            qs[(ch + 1) % 3].dma_start(
                out=xt[nh + nfin :, 4 * cols :], in_=xc[ch][:, 4 * cols :]
            )
            qs[(ch + 2) % 3].dma_start(
                out=xt[nh : nh + nfin, 4 * cols : 8 * cols], in_=fin[ch, :, 1, :]
            )

        ostages = [None] * c
        for t in range(t_steps + 1):
            for ch in range(c):
                xt = xtiles[ch]
                blk = t % 8
                p1 = psum.tile([mtot, cols], F32, tag=f"p{ch}", name=f"p_{ch}_{t}")
                nc.tensor.matmul(
                    p1, w0s if t == 0 else ws,
                    xt[:, blk * cols : (blk + 1) * cols], start=True, stop=True,
                )

                if t < t_steps:
                    # h_t goes straight into the next step's rhs block.
                    nb = (t + 1) % 8
                    nc.scalar.activation(
                        xt[0:nh, nb * cols : (nb + 1) * cols], p1[0:nh, :], tanh
                    )
                if t > 0:
                    ob = (t - 1) % 4
                    if ob == 0:
                        ostages[ch] = opool.tile(
                            [mout, 4 * cols], XDT, tag=f"o{ch}", name=f"o_{ch}_{t}"
                        )
                    ost = ostages[ch]
                    nc.vector.tensor_add(
                        ost[:, ob * cols : (ob + 1) * cols], p1[mpad:mtot, :], cbs[ch]
                    )
                    if ob == 3:
                        nc.gpsimd.dma_start(out=out[(t - 4) // 4, ch], in_=ost)
                # Prefetch fin group g = steps 4g..4g+3, ~5 steps ahead.
                if t % 4 == 3:
                    gg = (t + 5) // 4
                    if gg < nfg:
                        b0 = (4 * gg) % 8
                        nc.sync.dma_start(
                            out=xt[nh : nh + nfin, b0 * cols : (b0 + 4) * cols],
                            in_=fin[ch, :, gg, :],
                        )
    nc.compile()
    return nc


def build_packed_weights(W_rnn, W_out, b_rnn, b_out, g=G):
    W_rnn = np.asarray(W_rnn, np.float32)
    W_out = np.asarray(W_out, np.float32)
    b_rnn = np.asarray(b_rnn, np.float32)
    b_out = np.asarray(b_out, np.float32)
    W1p, W1f = W_rnn[:D_CP], W_rnn[D_CP:]
    Wo1, Wo2 = W_out[:D_CP], W_out[D_CP:]
    nh, nfin = HID * g, D_FIN * g
    ktot = nh + nfin + 1 + D_CP * g
    mpad, mtot = MPAD, MPAD + D_CP * g
    ones_row = nh + nfin
    cp0_base = ones_row + 1

    w = np.zeros((ktot, mtot), np.float32)
    w0 = np.zeros((ktot, mtot), np.float32)
    E = Wo1 @ W1p                      # (2, 12) cp0 contribution to next pre
    r = b_rnn + b_out @ W1p            # (12,) ones-row weight (steady state)
    Wh = Wo2 @ W1p                     # (12, 12) h contribution to next pre
    for i in range(g):
        hsl = slice(HID * i, HID * (i + 1))
        osl = slice(mpad + D_CP * i, mpad + D_CP * (i + 1))
        w[hsl, hsl] = Wh
        w[hsl, osl] = Wo2
        w0[hsl, osl] = Wo2             # unused at t=0 (h rows are zero) but harmless
        fsl = slice(nh + D_FIN * i, nh + D_FIN * (i + 1))
        w[fsl, hsl] = W1f
        w0[fsl, hsl] = W1f
        w[ones_row, hsl] = r
        w0[ones_row, hsl] = b_rnn
        csl = slice(cp0_base + D_CP * i, cp0_base + D_CP * (i + 1))
        w[csl, hsl] = E
        w0[csl, hsl] = W1p
    return w, w0


def stage_inputs(cp0, fin, cvec, g=G, c=C, cols=COLS, t_steps=T):
    """Batch-major -> feature-major device layouts (b = ch*(g*cols)+gi*cols+j)."""
    bp = g * c * cols
    bc = cp0.shape[0]
    nfg = (t_steps + 3) // 4 + 1
    F = np.zeros((bp, 4 * nfg, D_FIN), np.float32)
    F[:bc, :t_steps] = fin
    cp0_p = np.zeros((bp, D_CP), np.float32)
    cp0_p[:bc] = cp0
    cv_p = np.zeros((bp, D_CP), np.float32)
    cv_p[:bc] = cvec
    # fin group g covers steps 4g..4g+3, contiguous per partition row.
    fin_d = np.ascontiguousarray(
        F.reshape(c, g, cols, nfg, 4, D_FIN).transpose(0, 1, 5, 3, 4, 2)
    ).reshape(c, D_FIN * g, nfg, 4 * cols)
    xc_d = np.ones((c, 1 + D_CP * g, cols), np.float32)
    xc_d[:, 1:, :] = cp0_p.reshape(c, g, cols, D_CP).transpose(0, 1, 3, 2).reshape(
        c, D_CP * g, cols
    )
    xc_d = np.tile(xc_d, (1, 1, 8))
    cb_d = np.ascontiguousarray(
        cv_p.reshape(c, g, cols, D_CP).transpose(0, 1, 3, 2)
    ).reshape(c, D_CP * g, cols)
    return fin_d, xc_d, cb_d


def unstage_output(out_d, bc, g=G, c=C, cols=COLS, t_steps=T):
    """out_d [T/4, c, 14, 4*cols] (step-within-group along the row) -> batch."""
    bp = g * c * cols
    nog = t_steps // 4
    o = out_d.reshape(nog, c, g, D_CP, 4, cols).transpose(1, 2, 5, 0, 4, 3)
    return np.ascontiguousarray(o).reshape(bp, t_steps, D_CP)[:bc]


def kernel(control_point_input, finger_input, W_rnn, U_rnn, b_rnn, W_out, b_out):
    global LAST_RESULTS
    cp = np.asarray(control_point_input, np.float32)
    fin = np.asarray(finger_input, np.float32)
    W_rnn = np.asarray(W_rnn, np.float32)
    b_rnn = np.asarray(b_rnn, np.float32)
    W_out = np.asarray(W_out, np.float32)
    b_out = np.asarray(b_out, np.float32)

    cp0 = cp[:, 0, :]
    cvec = cp0 @ W_out[:D_CP] + b_out
    w, w0 = build_packed_weights(W_rnn, W_out, b_rnn, b_out)
    w, w0 = (x.astype(MM_NP) for x in (w, w0))

    nc = build_program()
    in_maps = []
    for m in range(NCORES):
        sl = slice(m * BC, (m + 1) * BC)
        fin_d, xc_d, cb_d = stage_inputs(cp0[sl], fin[sl], cvec[sl])
        in_maps.append(
            {"fin": fin_d.astype(MM_NP, copy=False),
             "xc": xc_d.astype(MM_NP, copy=False), "cb": cb_d, "w": w, "w0": w0}
        )

    trace = bool(os.environ.get("DTB_TRACE"))
    res = run_bass_kernel_spmd(
        nc, in_maps, core_ids=list(range(NCORES)), trace=trace
    )
    LAST_RESULTS = res

    outs = [
        unstage_output(np.asarray(res.results[m]["out"], np.float32), BC)
        for m in range(NCORES)
    ]
    return np.concatenate(outs, axis=0)
